# revision 33
# baseline (speedup 1.0000x reference)
"""Trainium2 Bass kernel for hetero-GNN (2x ResGatedGraphConv + segment-mean pooling + MLP).

Sharding: destination-node range per core; each core processes the edges whose
dst falls in its range. Host marshalling sorts each core's dst nodes by degree
and groups 128//EPN similar-degree nodes per "quad" with EPN edge positions per
node per 128-edge subtile (a node's edges split into chunks of EPN across the
quad's subtile chain). The scatter one-hot is then one of EPN CONSTANT block
matrices B_off (B[p, off + p//EPN] = 1) — no per-edge one-hot build on device,
and zero-padded edge columns contribute exactly 0.

Device per bucket (128 dst slots): skip-connection matmul opens a PSUM
accumulation, per-subtile fused transform matmul [x_src;ea;1;x_dst] @ W_aug
-> sigmoid (ACT) -> gated message (DVE) -> constant-B scatter matmul
accumulates into the same PSUM; relu (ACT) evacuates to SBUF. Segment-mean
pooling via one-hot matmul, cross-core AllReduce, 4-layer MLP head.
"""
import sys
import types
import numpy as np

NCORES = 8
G = 128
H = 64
F = 16
NC_N = 100000
NB_N = 200000
BUCKET = 128
GRP = 4  # subtile PAIRS per transform/sigmoid/mult group (8 subtiles)
LAST_EXEC_NS = None


def _install_ntff_shim():
    if 'antenv.axon_hooks' in sys.modules:
        return
    try:
        mod = types.ModuleType('antenv.axon_hooks')
        _h = [None]
        mod.set_axon_ntff_profile_hook = lambda h: _h.__setitem__(0, h)
        mod.get_axon_ntff_profile_hook = lambda: _h[0]
        sys.modules['antenv.axon_hooks'] = mod
        import antenv
        antenv.axon_hooks = mod
        from trn_agent_boot.trn_boot import _ntff_profile_via_ctypes
        mod.set_axon_ntff_profile_hook(
            _ntff_profile_via_ctypes('/opt/axon/libaxon_pjrt.so'))
    except Exception:
        pass


def _prep_relation(x_src, x_dst, src, dst, ea, D, epn):
    """Host marshalling for one relation (degree-grouped constant-scatter layout).

    Subtile structure (per-quad subtile counts) is shared across cores (max),
    so one compiled program serves all 8 cores SPMD. Subtiles are PAIR-PACKED
    for 2-way PE row-group packing: pair p holds subtile 2p's 35 xt rows at
    xtp[0:35] and subtile 2p+1's at xtp[35:70]. Each bucket's subtile count is
    padded to even (dummy zero subtiles scatter exactly 0).
    """
    quad = 128 // epn
    nbuck = (D + BUCKET - 1) // BUCKET
    Dp = nbuck * BUCKET
    nquad = Dp // quad
    qpb = BUCKET // quad
    xs = x_src.astype(np.float16)
    xd = x_dst.astype(np.float16)

    cores = []
    degs_all = np.zeros((NCORES, Dp), np.int64)
    for m in range(NCORES):
        lo, hi = m * D, (m + 1) * D
        mask = (dst >= lo) & (dst < hi)
        c_src, c_dst, c_ea = src[mask], dst[mask] - lo, ea[mask, 0]
        deg = np.bincount(c_dst, minlength=D)
        perm = np.argsort(-deg, kind="stable")
        inv = np.empty(D, np.int64)
        inv[perm] = np.arange(D)
        degs_all[m, :D] = deg[perm]
        cores.append((c_src, c_dst, c_ea, perm, inv))

    dmax_q = degs_all.max(axis=0).reshape(nquad, quad).max(axis=1)
    nsub_q = (dmax_q + epn - 1) // epn
    sub_start = np.zeros(nquad + 1, np.int64)
    sub_start[1:] = np.cumsum(nsub_q)
    # per-bucket: local subtile offsets (by quad), padded to even count
    bucket_npair = np.zeros(nbuck, np.int64)
    bucket_offs = []          # per bucket: list of off-index per subtile slot
    for b in range(nbuck):
        offs = []
        for q in range(qpb * b, min(qpb * b + qpb, nquad)):
            offs += [q % qpb] * int(nsub_q[q])
        if len(offs) % 2:
            offs.append(-1)   # dummy (no scatter matmul emitted)
        bucket_offs.append(offs)
        bucket_npair[b] = len(offs) // 2
    pair_start = np.zeros(nbuck + 1, np.int64)
    pair_start[1:] = np.cumsum(bucket_npair)
    npair_tot = int(pair_start[-1])
    ntot = npair_tot * 128
    per_core = []
    for m in range(NCORES):
        lo = m * D
        c_src, c_dst, c_ea, perm, inv = cores[m]
        s_of_edge = inv[c_dst]
        order = np.argsort(s_of_edge, kind="stable")
        se, srce, eae, dste = (s_of_edge[order], c_src[order], c_ea[order],
                               c_dst[order])
        rank = np.arange(len(se)) - np.searchsorted(se, se)
        q = se // quad
        t = rank // epn
        b_e = q // qpb                                    # bucket of edge
        l = sub_start[q] + t - sub_start[qpb * b_e]       # local subtile
        pair = pair_start[b_e] + l // 2
        half = l % 2
        col = pair * 128 + (se % quad) * epn + (rank % epn)

        xtp = np.zeros((70, ntot), np.float16)
        for hv in (0, 1):
            mk = half == hv
            c, o = col[mk], hv * 35
            xtp[o:o + 16, c] = xs[srce[mk]].T
            xtp[o + 16, c] = eae[mk].astype(np.float16)
            xtp[o + 17, c] = 1.0
            xtp[o + 18:o + 34, c] = xd[lo + dste[mk]].T

        pa = np.zeros((17, Dp), np.float16)
        pa[0:16, :D] = xd[lo + perm].T
        pa[16, :D] = 1.0
        perm_full = np.full(Dp, -1, np.int64)
        perm_full[:D] = perm
        per_core.append({"xt": xtp, "pa": pa, "perm": perm_full})

    return {"nbuck": nbuck, "Dp": Dp, "epn": epn, "quad": quad,
            "bucket_offs": bucket_offs, "pair_start": pair_start,
            "npair_tot": npair_tot, "per_core": per_core}


def _make_B(epn):
    quad = 128 // epn
    noff = BUCKET // quad
    B = np.zeros((128, noff * 128), np.float16)
    p = np.arange(128)
    for o in range(noff):
        B[p, o * 128 + o * quad + p // epn] = 1.0
    return B  # [128, noff*128]; lhsT slice o is B[:, o*128:(o+1)*128]


def _batch_layout(batch, perm_full, lo):
    """[128, nbuck] fp16 graph ids per slot (-1 ghosts)."""
    Dp = len(perm_full)
    bt = np.full(Dp, -1.0, np.float32)
    real = perm_full >= 0
    bt[real] = batch[lo + perm_full[real]].astype(np.float32)
    return np.ascontiguousarray(bt.reshape(-1, BUCKET).T)


def kernel(**inputs):
    _install_ntff_shim()
    import concourse.bass as bass  # noqa: F401
    import concourse.bacc as bacc
    import concourse.mybir as mybir
    import concourse.tile as tile
    from concourse.bass_utils import run_bass_kernel_spmd

    import ml_dtypes
    F32 = mybir.dt.float32
    F16 = mybir.dt.float16
    F8 = mybir.dt.float8e4
    NPF8 = np.dtype(ml_dtypes.float8_e4m3)
    AF = mybir.ActivationFunctionType
    OP = mybir.AluOpType

    ii = {k: np.asarray(v) for k, v in inputs.items()}
    Dc, Db = NC_N // NCORES, NB_N // NCORES

    rel_c = _prep_relation(ii["x_x"], ii["x_c"], ii["src_ac"].astype(np.int64),
                           ii["dst_ac"].astype(np.int64), ii["ea_ac"], Dc, 4)
    rel_b = _prep_relation(ii["x_c"], ii["x_b"], ii["src_cb"].astype(np.int64),
                           ii["dst_cb"].astype(np.int64), ii["ea_cb"], Db, 2)

    cnt_c = np.bincount(ii["batch_c"].astype(np.int64), minlength=G).astype(np.float32)
    cnt_b = np.bincount(ii["batch_b"].astype(np.int64), minlength=G).astype(np.float32)
    recip = np.stack([1.0 / np.maximum(cnt_c, 1.0),
                      1.0 / np.maximum(cnt_b, 1.0)]).astype(np.float16)  # [2, G]

    def waug(rel):
        Wq, Wv, Wk = ii[f"Wq_{rel}"], ii[f"Wv_{rel}"], ii[f"Wk_{rel}"]
        We = ii[f"We_{rel}"][0]
        bq, bv, bk, be = (ii[f"bq_{rel}"], ii[f"bv_{rel}"],
                          ii[f"bk_{rel}"], ii[f"be_{rel}"])
        w = np.zeros((35, 128), np.float32)
        w[0:16, 0:64] = Wq; w[0:16, 64:128] = Wv
        w[16, 0:64] = 2 * We; w[16, 64:128] = We
        w[17, 0:64] = bq + bk + 2 * be; w[17, 64:128] = bv + be
        w[18:34, 0:64] = Wk
        w2 = np.zeros((70, 256), np.float32)
        w2[0:35, 0:128] = w
        w2[35:70, 128:256] = w
        return w2.astype(np.float16)

    def wskip(rel):
        w = np.zeros((17, 64), np.float32)
        w[0:16] = ii[f"Wskip_{rel}"]
        w[16] = ii[f"bconv_{rel}"]
        return w.astype(np.float16)

    iota_g = np.tile(np.arange(G, dtype=np.float16), (128, 1))
    mlp_w = {
        "W1": ii["W1"].astype(np.float16), "W2": ii["W2"].astype(np.float16),
        "W3": ii["W3"].astype(np.float16), "Wout": ii["Wout"].astype(np.float16),
        "b1": ii["b1"].astype(np.float32).reshape(64, 1),
        "b2": ii["b2"].astype(np.float32).reshape(64, 1),
        "b3": ii["b3"].astype(np.float32).reshape(64, 1),
        "bout": ii["bout"].astype(np.float32).reshape(1, 1),
    }

    # ---------------- device program ----------------
    nc = bacc.Bacc("TRN2", target_bir_lowering=False, debug=False,
                   num_devices=NCORES)

    def din(name, arr0):
        return nc.dram_tensor(name, list(arr0.shape),
                              mybir.dt.from_np(arr0.dtype), kind="ExternalInput")

    B_c = _make_B(4).astype(NPF8)
    B_b = _make_B(2).astype(NPF8)
    h = {}
    h["xt_c"] = din("xt_c", rel_c["per_core"][0]["xt"])
    h["xt_b"] = din("xt_b", rel_b["per_core"][0]["xt"])
    h["pa_c"] = din("pa_c", rel_c["per_core"][0]["pa"])
    h["pa_b"] = din("pa_b", rel_b["per_core"][0]["pa"])
    bt_c0 = _batch_layout(ii["batch_c"], rel_c["per_core"][0]["perm"], 0)
    bt_b0 = _batch_layout(ii["batch_b"], rel_b["per_core"][0]["perm"], 0)
    h["bt_c"] = din("bt_c", bt_c0)
    h["bt_b"] = din("bt_b", bt_b0)
    h["B_c"] = din("B_c", B_c)
    h["B_b"] = din("B_b", B_b)
    h["waug_c"] = din("waug_c", waug("ac"))
    h["waug_b"] = din("waug_b", waug("cb"))
    h["wskip_c"] = din("wskip_c", wskip("ac"))
    h["wskip_b"] = din("wskip_b", wskip("cb"))
    h["iotag"] = din("iotag", iota_g)
    h["recip"] = din("recip", recip)
    sel2 = np.zeros((2, 128), np.float16); sel2[0, 0:64] = 1; sel2[1, 64:128] = 1
    h["ones2"] = din("ones2", sel2)
    for k, v in mlp_w.items():
        h["mlp_" + k] = din("mlp_" + k, v)
    out_h = nc.dram_tensor("out", [1, G], F32, kind="ExternalOutput")

    with tile.TileContext(nc) as tc:
        with tc.tile_pool(name="const", bufs=1) as cp, \
             tc.tile_pool(name="acc", bufs=1) as accp, \
             tc.tile_pool(name="stream", bufs=6) as sp, \
             tc.tile_pool(name="work", bufs=6) as wp, \
             tc.tile_pool(name="psum", bufs=2, space="PSUM") as pp, \
             tc.tile_pool(name="psb", bufs=3, space="PSUM") as ppB, \
             tc.tile_pool(name="psA", bufs=1, space="PSUM") as ppA, \
             tc.tile_pool(name="dram", bufs=1, space="DRAM") as dp:

            iotag_t = cp.tile([128, G], F16, tag="iotag_t")
            nc.sync.dma_start(iotag_t[:], h["iotag"].ap())
            pooled_ps = ppA.tile([128, G], F32, tag="pooled_ps")

            def relation(tag, rel, row_off):
                nbuck = rel["nbuck"]
                noff = BUCKET // rel["quad"]
                w_t = cp.tile([70, 256], F16, name=f"waug_{tag}", tag=f"waug_{tag}")
                nc.sync.dma_start(w_t[:], h[f"waug_{tag}"].ap())
                ws_t = cp.tile([17, 64], F16, name=f"wskip_{tag}", tag=f"wskip_{tag}")
                nc.sync.dma_start(ws_t[:], h[f"wskip_{tag}"].ap())
                B_t = cp.tile([128, noff, 128], F8, name=f"B_{tag}", tag=f"B_{tag}")
                nc.sync.dma_start(B_t[:], h[f"B_{tag}"].ap())
                pa_sb = accp.tile([17, rel["Dp"]], F16, name=f"pa_{tag}",
                                  tag=f"pa_{tag}")
                nc.sync.dma_start(pa_sb[:], h[f"pa_{tag}"].ap())
                bt_sb = accp.tile([128, nbuck], F32, name=f"bt_{tag}",
                                  tag=f"bt_{tag}")
                nc.sync.dma_start(bt_sb[:], h[f"bt_{tag}"].ap())
                h_sb = accp.tile([128, nbuck * 64], F16, name=f"h_{tag}",
                                 tag=f"h_{tag}")

                xt_v = h[f"xt_{tag}"].ap()
                pair_start = rel["pair_start"]
                # flat group list for software pipelining
                groups = []
                for b in range(nbuck):
                    p0 = int(pair_start[b])
                    npair = int(pair_start[b + 1]) - p0
                    offs = rel["bucket_offs"][b]
                    if npair == 0:
                        groups.append((b, p0, 0, 0, offs, True, True))
                        continue
                    s = 0
                    while s < npair:
                        g = min(GRP, npair - s)
                        groups.append((b, p0, s, g, offs, s == 0,
                                       s + g == npair))
                        s += g
                SKEW = 3
                sv_live = {}
                bps_live = {}
                for i in range(len(groups) + SKEW):
                    if i < len(groups):
                        b, p0, s, g, offs, first, last = groups[i]
                        if g > 0:
                            e0 = (p0 + s) * 128
                            xt_t = sp.tile([70, GRP * 128], F16,
                                           name=f"xt_{tag}_{i}", tag="xt")
                            nc.sync.dma_start(xt_t[:, :g * 128],
                                              xt_v[:, e0:e0 + g * 128])
                            # one block-diagonal matmul per pair:
                            # [xtA;xtB] @ [[w,0],[0,w]] -> [svA | svB]
                            sv = pp.tile([128, 2 * GRP * 128], F32,
                                         name=f"sv_{tag}_{i}", tag="sv")
                            for j in range(g):
                                nc.tensor.matmul(
                                    sv[:, (2 * j) * 128:(2 * j + 2) * 128],
                                    xt_t[:, j * 128:(j + 1) * 128],
                                    w_t[:], start=True, stop=True)
                            sv_live[i] = sv
                    jj = i - SKEW
                    if jj < 0 or jj >= len(groups):
                        continue
                    b, p0, s, g, offs, first, last = groups[jj]
                    cnt = len(offs)
                    if first:
                        bps = ppB.tile([128, 64], F32, name=f"bps_{tag}_{b}",
                                      tag="bps")
                        bps_live[b] = bps
                        nc.tensor.matmul(bps[:], pa_sb[:, b * 128:(b + 1) * 128],
                                         ws_t[:], start=True, stop=(cnt == 0),
                                         skip_group_check=True)
                    bps = bps_live[b]
                    if g > 0:
                        sv = sv_live.pop(jj)
                        sv3 = sv[:].rearrange("p (a b) -> p a b", a=2 * GRP)
                        gt = wp.tile([128, 2 * GRP, 64], F16,
                                     name=f"gt_{tag}_{jj}", tag="gt")
                        msg = wp.tile([128, 2 * GRP, 64], F8,
                                      name=f"msg_{tag}_{jj}", tag="msg")
                        nc.scalar.activation(gt[:, :2 * g, :],
                                             sv3[:, :2 * g, 0:64], AF.Sigmoid)
                        nc.vector.tensor_tensor(msg[:, :2 * g, :],
                                                gt[:, :2 * g, :],
                                                sv3[:, :2 * g, 64:128],
                                                op=OP.mult)
                        last_real = max(k for k in range(cnt)
                                        if offs[k] >= 0)
                        for li in range(2 * g):
                            gi = s * 2 + li
                            if offs[gi] < 0:
                                continue
                            nc.tensor.matmul(bps[:], B_t[:, offs[gi], :],
                                             msg[:, li, :], start=False,
                                             stop=(gi == last_real),
                                             skip_group_check=True)
                    if last:
                        nc.scalar.activation(h_sb[:, b * 64:(b + 1) * 64],
                                             bps[:], AF.Relu)
                        del bps_live[b]

                # deferred pooling for this relation: emitted after the
                # main loop so the next relation's work can overlap it
                for b in range(nbuck):
                    ohg = wp.tile([128, G], F16, name=f"ohg_{tag}_{b}",
                                  tag="ohg")
                    nc.vector.tensor_scalar(ohg[:], iotag_t[:],
                                            bt_sb[:, b:b + 1],
                                            None, OP.is_equal)
                    nc.tensor.matmul(pooled_ps[row_off:row_off + 64, :],
                                     h_sb[:, b * 64:(b + 1) * 64], ohg[:],
                                     start=(b == 0), stop=(b == nbuck - 1),
                                     skip_group_check=True)

            relation("c", rel_c, 0)
            relation("b", rel_b, 64)

            pooled_sb = accp.tile([128, G], F32, tag="pooled_sb")
            nc.vector.tensor_copy(pooled_sb[:], pooled_ps[:])
            bounce_in = dp.tile([128, G], F32, tag="bounce_in")
            bounce_out = dp.tile([128, G], F32, tag="bounce_out")
            nc.sync.dma_start(bounce_in[:], pooled_sb[:])
            nc.gpsimd.collective_compute(
                "AllReduce", OP.add, replica_groups=[list(range(NCORES))],
                ins=[bounce_in.opt()], outs=[bounce_out.opt()])
            nc.sync.dma_start(pooled_sb[:], bounce_out[:])

            recip_sb = accp.tile([2, G], F16, tag="recip_sb")
            nc.sync.dma_start(recip_sb[:], h["recip"].ap())
            ones2_sb = accp.tile([2, 128], F16, tag="ones2_sb")
            nc.sync.dma_start(ones2_sb[:], h["ones2"].ap())
            rb_ps = ppA.tile([128, G], F32, tag="pooled_ps")
            nc.tensor.matmul(rb_ps[:], ones2_sb[:], recip_sb[:],
                             start=True, stop=True)
            mean_sb = accp.tile([128, G], F16, tag="mean_sb")
            nc.vector.tensor_tensor(mean_sb[:], pooled_sb[:], rb_ps[:], op=OP.mult)

            mw, mb = {}, {}
            for k in ("W1", "W2", "W3", "Wout"):
                mw[k] = accp.tile(list(mlp_w[k].shape), F16, name=f"mw{k}",
                                  tag=f"mw{k}")
                nc.sync.dma_start(mw[k][:], h["mlp_" + k].ap())
            for k in ("b1", "b2", "b3", "bout"):
                mb[k] = accp.tile(list(mlp_w[k].shape), F32, name=f"mb{k}",
                                  tag=f"mb{k}")
                nc.sync.dma_start(mb[k][:], h["mlp_" + k].ap())

            hcur = mean_sb
            for li, (wk, bk) in enumerate((("W1", "b1"), ("W2", "b2"),
                                           ("W3", "b3"))):
                ps = ppA.tile([64, G], F32, name=f"mlp{li}", tag="pooled_ps")
                nc.tensor.matmul(ps[:], mw[wk][:], hcur[:], start=True, stop=True)
                hn = accp.tile([64, G], F16, name=f"hn{li}", tag=f"hn{li}")
                nc.scalar.activation(hn[:], ps[:], AF.Relu, bias=mb[bk][:])
                hcur = hn
            ps_o = ppA.tile([1, G], F32, tag="pooled_ps")
            nc.tensor.matmul(ps_o[:], mw["Wout"][:], hcur[:], start=True, stop=True)
            osb = accp.tile([1, G], F32, tag="osb")
            nc.scalar.activation(osb[:], ps_o[:], AF.Identity, bias=mb["bout"][:])
            nc.sync.dma_start(out_h.ap(), osb[:])

    nc.compile()

    in_maps = []
    for m in range(NCORES):
        in_maps.append({
            "xt_c": rel_c["per_core"][m]["xt"],
            "xt_b": rel_b["per_core"][m]["xt"],
            "pa_c": rel_c["per_core"][m]["pa"],
            "pa_b": rel_b["per_core"][m]["pa"],
            "bt_c": _batch_layout(ii["batch_c"], rel_c["per_core"][m]["perm"],
                                  m * Dc),
            "bt_b": _batch_layout(ii["batch_b"], rel_b["per_core"][m]["perm"],
                                  m * Db),
            "B_c": B_c, "B_b": B_b,
            "waug_c": waug("ac"), "waug_b": waug("cb"),
            "wskip_c": wskip("ac"), "wskip_b": wskip("cb"),
            "iotag": iota_g, "recip": recip, "ones2": sel2,
            **{"mlp_" + k: v for k, v in mlp_w.items()},
        })
    import os
    trace = bool(os.environ.get("KERNEL_TRACE"))
    res = run_bass_kernel_spmd(nc, in_maps, core_ids=list(range(NCORES)),
                               trace=trace)
    global LAST_EXEC_NS
    LAST_EXEC_NS = res.exec_time_ns
    return res.results[0]["out"].reshape(G).astype(np.float32)
